# revision 6
# baseline (speedup 1.0000x reference)
"""
LocallyConnected1d (kernel_size=1) Trainium2 Bass kernel.

Math:  out[b, oc, w] = sum_c x[b, c, w] * weights[w, oc, c, 0, 0]
Shapes: x (256, 384, 1024) f32, weights (1024, 384, 384, 1, 1) f32,
        out (256, 384, 1024) f32.

Strategy:
  - Width-shard W=1024 across 8 NeuronCores (128 positions each); zero
    communication (each w position is an independent (B,C)@(C,OC) matmul).
  - Host-side: cast x/weights to bf16 (RNE bit trick) and rearrange each
    shard into DMA-friendly layouts:
        x    -> (C, Ws, B)   bf16
        w    -> (C, Ws, OC)  bf16
        out  <- (B, Ws, OC)  f32 (device-written), host transposes back.
  - Device: per w position, 2 (b-tile) x 3 (c-tile) matmuls with c on the
    partition (contraction) dim: psum[b(128), oc(384)] += xT.T @ wT,
    accumulated over 3 k-tiles in PSUM, copied to SBUF by DVE, DMA'd out.
  - bf16 inputs halve HBM traffic (the kernel is HBM-bound); accumulation
    is fp32 in PSUM and the output is fp32.
"""

import os
import sys
from concurrent.futures import ThreadPoolExecutor

import numpy as np

for _p in ("/opt/trn_rl_repo", os.path.expanduser("~/.axon_site/_ro/trn_rl_repo")):
    if os.path.isdir(_p) and _p not in sys.path:
        sys.path.insert(0, _p)
        break

import ml_dtypes

_BF16 = ml_dtypes.bfloat16

# Problem shapes (hardcoded per the contract).
B, C, OC, W = 256, 384, 384, 1024
NCORES = 8
WS = W // NCORES  # 128 width positions per core
KO = C // 128     # 3 contraction (c) tiles of 128
BO = B // 128     # 2 batch tiles of 128

# Device kernel tiling: w positions per SBUF block.
WBLK = 8
NBLK = WS // WBLK

LAST_RESULTS = None   # BassKernelResults of the most recent run (for profiling)
_NC_CACHE = {}


def _build_nc(b=B, c=C, oc=OC, ws=WS, wblk=WBLK, repeat=1):
    """Build the SPMD Tile program (identical on every core)."""
    import concourse.bacc as bacc
    import concourse.mybir as mybir
    import concourse.tile as tile

    ko = c // 128
    bo_n = b // 128
    nblk = ws // wblk

    # Bacc (not bass.Bass): its generate_event_semaphores pass splits
    # multi-sem waits, which the TRN2 ISA requires (walrus errors with
    # "Too many sync wait commands" otherwise).
    nc = bacc.Bacc("TRN2", target_bir_lowering=False, debug=False)

    x_ext = nc.dram_tensor("x", [c, ws, b], mybir.dt.bfloat16, kind="ExternalInput")
    w_ext = nc.dram_tensor("w", [c, ws, oc], mybir.dt.bfloat16, kind="ExternalInput")
    o_ext = nc.dram_tensor("out", [b, ws, oc], mybir.dt.float32, kind="ExternalOutput")

    x_r = x_ext.ap().rearrange("(ko ki) w b -> ki ko w b", ki=128)
    w_r = w_ext.ap().rearrange("(ko ki) w oc -> ki ko w oc", ki=128)
    o_r = o_ext.ap().rearrange("(bo bi) w oc -> bi bo w oc", bi=128)

    with tile.TileContext(nc) as tc:
        with (
            tc.tile_pool(name="xp", bufs=3) as xp,
            tc.tile_pool(name="wp", bufs=3) as wp,
            tc.tile_pool(name="op", bufs=3) as op,
            tc.tile_pool(name="pp", bufs=8, space="PSUM") as pp,
        ):
            for blk in [i for _ in range(repeat) for i in range(nblk)]:
                wsl = slice(blk * wblk, (blk + 1) * wblk)
                xt = xp.tile([128, ko, wblk, b], mybir.dt.bfloat16)
                nc.sync.dma_start(xt[:], x_r[:, :, wsl, :])
                wt = wp.tile([128, ko, wblk, oc], mybir.dt.bfloat16)
                nc.sync.dma_start(wt[:], w_r[:, :, wsl, :])
                ot = op.tile([128, bo_n, wblk, oc], mybir.dt.float32)
                for wi in range(wblk):
                    for bo in range(bo_n):
                        ps = pp.tile([128, oc], mybir.dt.float32)
                        for k in range(ko):
                            nc.tensor.matmul(
                                ps,
                                xt[:, k, wi, bo * 128:(bo + 1) * 128],
                                wt[:, k, wi, :],
                                start=(k == 0),
                                stop=(k == ko - 1),
                            )
                        nc.vector.tensor_copy(out=ot[:, bo, wi, :], in_=ps)
                # Output DMAs go on the scalar-engine HWDGE ring so they
                # don't head-of-line-block the input loads on sync.
                nc.scalar.dma_start(o_r[:, :, wsl, :], ot[:])
    nc.finalize()  # runs Bacc passes: reg alloc + TRN2 sem-wait splitting
    return nc


def _get_nc():
    key = (B, C, OC, WS, WBLK)
    if key not in _NC_CACHE:
        _NC_CACHE[key] = _build_nc()
    return _NC_CACHE[key]


def _to_bf16_bits(a):
    """fp32 ndarray -> uint16 bf16 bits, round-to-nearest-even."""
    u = np.ascontiguousarray(a).view(np.uint32)
    r = ((u >> np.uint32(16)) & np.uint32(1)) + np.uint32(0x7FFF)
    return ((u + r) >> np.uint32(16)).astype(np.uint16)


def _t2_blocked(a):
    """Cache-blocked 2D transpose of a contiguous (R, S) array, R%32==S%32==0."""
    r, s = a.shape
    a4 = a.reshape(r // 32, 32, s // 32, 32)
    return np.ascontiguousarray(a4.transpose(2, 3, 0, 1)).reshape(s, r)


def _prep_x_shard(x_bits, s):
    # x_bits (B, C, W) u16 -> shard (C, Ws, B) u16
    xs = np.ascontiguousarray(x_bits[:, :, s * WS:(s + 1) * WS])
    return _t2_blocked(xs.reshape(B, C * WS)).reshape(C, WS, B).view(_BF16)

def _prep_w_shard(w_bits, s):
    # w_bits (W, OC, C) u16 -> shard (C, Ws, OC) u16
    slab = w_bits[s * WS:(s + 1) * WS]                      # (Ws, OC, C)
    t = np.ascontiguousarray(slab.transpose(0, 2, 1))       # (Ws, C, OC) L1-friendly
    return np.ascontiguousarray(t.transpose(1, 0, 2)).view(_BF16)  # (C, Ws, OC) chunk memcpy


def kernel(x, weights):
    from concourse.bass_utils import run_bass_kernel_spmd

    global LAST_RESULTS

    x = np.ascontiguousarray(x, dtype=np.float32)
    wsq = np.ascontiguousarray(weights, dtype=np.float32).reshape(W, OC, C)

    x_bits = np.empty(x.shape, np.uint16)
    w_bits = np.empty(wsq.shape, np.uint16)
    with ThreadPoolExecutor(max_workers=NCORES) as ex:
        futs = []
        for i in range(NCORES):
            bsl = slice(i * (B // NCORES), (i + 1) * (B // NCORES))
            wsl = slice(i * WS, (i + 1) * WS)
            futs.append(ex.submit(
                lambda bsl=bsl: x_bits.__setitem__(bsl, _to_bf16_bits(x[bsl]))))
            futs.append(ex.submit(
                lambda wsl=wsl: w_bits.__setitem__(wsl, _to_bf16_bits(wsq[wsl]))))
        for f in futs:
            f.result()
        x_shards = list(ex.map(lambda s: _prep_x_shard(x_bits, s), range(NCORES)))
        w_shards = list(ex.map(lambda s: _prep_w_shard(w_bits, s), range(NCORES)))

    nc = _get_nc()
    in_maps = [{"x": x_shards[s], "w": w_shards[s]} for s in range(NCORES)]
    res = run_bass_kernel_spmd(nc, in_maps, list(range(NCORES)))
    LAST_RESULTS = res

    out = np.empty((B, OC, W), np.float32)

    def _fin(s):
        out[:, :, s * WS:(s + 1) * WS] = res.results[s]["out"].transpose(0, 2, 1)

    with ThreadPoolExecutor(max_workers=NCORES) as ex:
        list(ex.map(_fin, range(NCORES)))
    return out


def measure_hw_ns(r_lo=1, r_hi=5):
    """Estimate pure HW exec time without NTFF profiling: time the second
    (compile-cached) run of a 1x-body vs Rx-body kernel; the delta divided by
    (r_hi - r_lo) cancels transfer/dispatch overhead."""
    import time
    from concourse.bass_utils import run_bass_kernel_spmd

    rng = np.random.default_rng(0)
    xs = rng.standard_normal((C, WS, B), dtype=np.float32).astype(_BF16)
    ws_ = rng.random((C, WS, OC), dtype=np.float32).astype(_BF16)
    in_maps = [{"x": xs, "w": ws_} for _ in range(NCORES)]

    times = {}
    for r in (r_lo, r_hi):
        nc = _build_nc(repeat=r)
        run_bass_kernel_spmd(nc, in_maps, list(range(NCORES)))  # compile+warm
        t0 = time.time()
        run_bass_kernel_spmd(nc, in_maps, list(range(NCORES)))
        times[r] = time.time() - t0
    return (times[r_hi] - times[r_lo]) / (r_hi - r_lo) * 1e9, times
